# revision 14
# baseline (speedup 1.0000x reference)
"""Distributed Trainium2 kernel for: a = x.T @ x ; b = softmax(a, axis=0) ; c = x @ b.

Strategy (8 NeuronCores, no collectives — embarrassingly parallel column shard):
  Core i owns output columns S_i = [512*i, 512*(i+1)).
  Since a is symmetric, the column-softmax stats for columns S_i are the row
  stats of the row shard a[S_i, :], which reduce along the free axis on-chip.

  Phase 1: a_S = x[:, S].T @ x          [512, 4096]   (Gram row-shard, f32 PSUM)
  Phase 2: P = row_softmax(a_S)         (= b[:, S].T, computed in f32)
  Phase 3: PE-transpose P -> b_S        [4096, 512]
  Phase 4: c[:, S] = x @ b_S            via lhsT = x.T tiles (host-pretiled)

Matmul operands are bf16 (1 cycle/row on the PE — 4-byte fp32 operands stream
at half rate) with fp32 PSUM accumulation; the softmax stats run in fp32.
"""

import numpy as np

N, D, P = 8192, 4096, 128
NCORES = 8
JS = D // NCORES          # 512 columns per core
SBI = JS // P             # 4 shard row-blocks of a_S
NKT = N // P              # 64 contraction tiles for the Gram
NCH = D // JS             # 8 chunks of 512 over the Gram free dim
DKT = D // P              # 32 contraction tiles for phase 4
NB = N // P               # 64 output row blocks

_nc_cache = None


def _build():
    import concourse.bass as bass
    import concourse.mybir as mybir
    import concourse.tile as tile
    from concourse import bacc
    from concourse.masks import make_identity

    f32 = mybir.dt.float32
    bf16 = mybir.dt.bfloat16
    fp8 = mybir.dt.float8e4

    nc = bacc.Bacc("TRN2", target_bir_lowering=False)
    # fp8 e4m3 copies of x feed the Gram phase (DoubleRow, 2x MACs/cycle);
    # the Gram only feeds a saturated softmax, so fp8 precision is ample.
    x8 = nc.dram_tensor("x8", (N, D), fp8, kind="ExternalInput")
    xs8 = nc.dram_tensor("xs8", (N, JS), fp8, kind="ExternalInput")
    # xtl[nb, p, kt, n] = x[nb*128 + n, kt*128 + p] — phase-4 lhsT tiles, one
    # fully contiguous 1 MiB DMA per output row-block.
    xtl = nc.dram_tensor("xtl", (NB, P, DKT, P), bf16, kind="ExternalInput")
    out = nc.dram_tensor("out", (N, JS), f32, kind="ExternalOutput")
    # scratch for relaying 1/rowsum from partition layout to free-axis layout
    rsd = nc.dram_tensor("rsd", (SBI, P), f32)

    with tile.TileContext(nc) as tc:
        with (
            tc.tile_pool(name="psum", bufs=8, space="PSUM") as psum,
            tc.tile_pool(name="stats", bufs=8) as stats,
            tc.tile_pool(name="singles", bufs=1) as singles,
            tc.tile_pool(name="ptp", bufs=DKT) as ptp,
        ):
            ident = singles.tile([P, P], bf16)
            make_identity(nc, ident)
            pt = [ptp.tile([P, JS], bf16, tag="pt", name=f"pt{i}") for i in range(DKT)]

            with (
                tc.tile_pool(name="big", bufs=5) as big,
                tc.tile_pool(name="xsp", bufs=NKT // 2) as xsp,
                tc.tile_pool(name="rhsp", bufs=6) as rhsp,
                tc.tile_pool(name="xtp", bufs=5) as xtp,
                tc.tile_pool(name="outp", bufs=3) as outp,
            ):
                a_s = [
                    big.tile([P, D], f32, tag="big", name=f"a_s{i}")
                    for i in range(SBI)
                ]
                pmax = [
                    stats.tile([P, NCH], f32, tag="pmax", name=f"pmax{i}", bufs=4)
                    for i in range(SBI)
                ]
                if True:
                    # ---------------- Phase 1: Gram row-shard ----------------
                    # fp8 DoubleRow: each matmul contracts a k-PAIR of 128-row
                    # tiles (virtual 128x256 array, 2 fp8 weights per cell).
                    # One explicit LDWEIGHTS per (k-pair, bi) serves TWO
                    # matmuls (1024-wide d2 pass-groups, 8 PSUM banks =
                    # 4 bi x 2 halves) so the 256-column DoubleRow weight
                    # load hides under the matmul stream.
                    NKP = NKT // 2
                    NPG = 4  # d2 pass-groups of 1024
                    xst = [
                        xsp.tile([P, 2, JS], fp8, tag="xs", name=f"xs_{k}")
                        for k in range(NKP)
                    ]
                    last_pss = [None] * SBI
                    for pg in range(NPG):
                        pss = [
                            psum.tile([P, JS], f32, tag="ps", name=f"ps1_{pg}_{i}")
                            for i in range(2 * SBI)
                        ]
                        g0 = pg * 2 * JS
                        for kp in range(NKP):
                            r0 = kp * 2 * P
                            if pg == 0:
                                nc.gpsimd.dma_start(
                                    out=xst[kp],
                                    in_=xs8[r0 : r0 + 2 * P, :].rearrange(
                                        "(ko p) m -> p ko m", p=P
                                    ),
                                )
                            rt = rhsp.tile(
                                [P, 2, 2 * JS], fp8, tag="rt", name=f"rt_{pg}_{kp}"
                            )
                            nc.sync.dma_start(
                                out=rt,
                                in_=x8[r0 : r0 + 2 * P, g0 : g0 + 2 * JS].rearrange(
                                    "(ko p) d -> p ko d", p=P
                                ),
                            )
                            for bi in range(SBI):
                                w_ap = xst[kp][:, :, bi * P : (bi + 1) * P]
                                with tc.tile_critical():
                                    nc.tensor.ldweights(
                                        w_ap,
                                        perf_mode=mybir.MatmulPerfMode.DoubleRow,
                                    )
                                    for h in range(2):
                                        mmi = nc.tensor.matmul(
                                            pss[bi * 2 + h],
                                            w_ap,
                                            rt[:, :, h * JS : (h + 1) * JS],
                                            start=(kp == 0),
                                            stop=(kp == NKP - 1),
                                            perf_mode=mybir.MatmulPerfMode.DoubleRow,
                                        )
                                        mmi.ins.ldweights = False
                        for bi in range(SBI):
                            for h in range(2):
                                ch = pg * 2 + h
                                nc.vector.reduce_max(
                                    out=pmax[bi][:, ch : ch + 1],
                                    in_=pss[bi * 2 + h],
                                    axis=mybir.AxisListType.X,
                                )
                        for bi in range(SBI):
                            for h in range(2):
                                ch = pg * 2 + h
                                if ch < NCH - 1:
                                    nc.vector.tensor_copy(
                                        out=a_s[bi][:, ch * JS : (ch + 1) * JS],
                                        in_=pss[bi * 2 + h],
                                    )
                                else:
                                    last_pss[bi] = pss[bi * 2 + h]

                # ------------- Phase 2+3: softmax rows, transpose -------------
                # exp is chunked so PE transposes chase the ACT engine instead
                # of waiting for whole rows; the 1/rowsum scale is deferred to
                # the phase-4 PSUM evacuation (column scales commute through
                # the matmul, and applying them in f32 at the end is exact).
                TPC = JS // P  # transposes per exp chunk
                # prefetch the first phase-4 lhsT blocks; the in-order sync
                # queue starts these the moment phase 1's stream drains, so
                # they land during the softmax/transposes.
                xtts = {}
                for nb in range(4):
                    xtts[nb] = xtp.tile([P, DKT, P], bf16, tag="xt", name=f"xtt{nb}")
                    nc.sync.dma_start(out=xtts[nb], in_=xtl[nb])
                for bi in range(SBI):
                    m = stats.tile([P, 1], f32, tag="m", name=f"m{bi}")
                    nc.vector.reduce_max(out=m, in_=pmax[bi], axis=mybir.AxisListType.X)
                    negm = stats.tile([P, 1], f32, tag="negm", name=f"negm{bi}")
                    nc.vector.tensor_scalar_mul(out=negm, in0=m, scalar1=-1.0)
                    pacc = stats.tile([P, NCH], f32, tag="pacc", name=f"pacc{bi}", bufs=4)
                    p_s = big.tile([P, D], bf16, tag="big", name=f"p_s{bi}")
                    for c in [NCH - 1] + list(range(NCH - 1)):
                        c0 = c * JS
                        src_ap = (
                            last_pss[bi] if c == NCH - 1 else a_s[bi][:, c0 : c0 + JS]
                        )
                        nc.scalar.activation(
                            out=p_s[:, c0 : c0 + JS],
                            in_=src_ap,
                            func=mybir.ActivationFunctionType.Exp,
                            bias=negm,
                            scale=1.0,
                            accum_out=pacc[:, c : c + 1],
                        )
                        for t in range(c * TPC, (c + 1) * TPC):
                            tp = psum.tile([P, P], bf16, tag="ps", name=f"tp{bi}_{t}")
                            nc.tensor.transpose(tp, p_s[:, t * P : (t + 1) * P], ident)
                            nc.vector.tensor_copy(
                                out=pt[t][:, bi * P : (bi + 1) * P], in_=tp
                            )
                    ssum = stats.tile([P, 1], f32, tag="ssum", name=f"ssum{bi}")
                    nc.vector.reduce_sum(out=ssum, in_=pacc, axis=mybir.AxisListType.X)
                    rs = stats.tile([P, 1], f32, tag="rs", name=f"rs{bi}")
                    nc.vector.reciprocal(out=rs, in_=ssum)
                    nc.gpsimd.dma_start(out=rsd[bi], in_=rs)
                # broadcast [512] reciprocals across partitions: [128, SBI*P]
                rsb = singles.tile([P, SBI, P], f32, name="rsb")
                nc.gpsimd.dma_start(
                    out=rsb,
                    in_=bass.AP(tensor=rsd, offset=0, ap=[[0, P], [P, SBI], [1, P]]),
                )

                # ---------------- Phase 4: c_S = x @ b_S ----------------
                for nb in range(NB):
                    if nb in xtts:
                        xtt = xtts.pop(nb)
                    else:
                        xtt = xtp.tile([P, DKT, P], bf16, tag="xt", name=f"xtt{nb}")
                        nc.sync.dma_start(out=xtt, in_=xtl[nb])
                    ps = psum.tile([P, JS], f32, tag="ps", name=f"ps4_{nb}")
                    for kt in range(DKT):
                        nc.tensor.matmul(
                            ps,
                            xtt[:, kt, :],
                            pt[kt],
                            start=(kt == 0),
                            stop=(kt == DKT - 1),
                        )
                    ot = outp.tile([P, JS], f32, tag="ot", name=f"ot{nb}")
                    nc.vector.tensor_mul(
                        out=ot, in0=ps, in1=rsb.rearrange("p a b -> p (a b)")
                    )
                    nc.sync.dma_start(out=out[nb * P : (nb + 1) * P, :], in_=ot)
    nc.finalize()
    return nc


def _get_nc():
    global _nc_cache
    if _nc_cache is None:
        _nc_cache = _build()
    return _nc_cache


def kernel(x):
    import ml_dtypes
    from concourse.bass_utils import run_bass_kernel_spmd

    x = np.asarray(x, dtype=np.float32)
    assert x.shape == (N, D)
    xb = x.astype(ml_dtypes.bfloat16)
    x8 = x.astype(ml_dtypes.float8_e4m3)
    # xtl[nb, p, kt, n] = x[nb*128 + n, kt*128 + p]
    xtl = np.ascontiguousarray(
        xb.reshape(NB, P, DKT, P).transpose(0, 3, 2, 1)
    )
    in_maps = [
        {
            "x8": x8,
            "xs8": np.ascontiguousarray(x8[:, i * JS : (i + 1) * JS]),
            "xtl": xtl,
        }
        for i in range(NCORES)
    ]
    nc = _get_nc()
    res = run_bass_kernel_spmd(nc, in_maps, core_ids=list(range(NCORES)))
    out = np.concatenate([r["out"] for r in res.results], axis=1)
    return out


# revision 16
# speedup vs baseline: 2.6318x; 2.6318x over previous
"""Distributed Trainium2 kernel for: a = x.T @ x ; b = softmax(a, axis=0) ; c = x @ b.

Strategy (8 NeuronCores, no collectives — embarrassingly parallel column shard):
  Core i owns output columns S_i = [512*i, 512*(i+1)).
  Since a is symmetric, the column-softmax stats for columns S_i are the row
  stats of the row shard a[S_i, :], which reduce along the free axis on-chip.

  Phase 1: a_S = x[:, S].T @ x          [512, 4096]   (Gram row-shard, f32 PSUM)
  Phase 2: P = row_softmax(a_S)         (= b[:, S].T, computed in f32)
  Phase 3: PE-transpose P -> b_S        [4096, 512]
  Phase 4: c[:, S] = x @ b_S            via lhsT = x.T tiles (host-pretiled)

Matmul operands are bf16 (1 cycle/row on the PE — 4-byte fp32 operands stream
at half rate) with fp32 PSUM accumulation; the softmax stats run in fp32.
"""

import numpy as np

N, D, P = 8192, 4096, 128
NCORES = 8
JS = D // NCORES          # 512 columns per core
SBI = JS // P             # 4 shard row-blocks of a_S
NKT = N // P              # 64 contraction tiles for the Gram
NCH = D // JS             # 8 chunks of 512 over the Gram free dim
DKT = D // P              # 32 contraction tiles for phase 4
NB = N // P               # 64 output row blocks

_nc_cache = None


def _build():
    import concourse.bass as bass
    import concourse.mybir as mybir
    import concourse.tile as tile
    from concourse import bacc
    from concourse.masks import make_identity

    f32 = mybir.dt.float32
    bf16 = mybir.dt.bfloat16
    fp8 = mybir.dt.float8e4

    nc = bacc.Bacc("TRN2", target_bir_lowering=False)
    # fp8 e4m3 copies of x feed the Gram phase (DoubleRow, 2x MACs/cycle);
    # the Gram only feeds a saturated softmax, so fp8 precision is ample.
    x8 = nc.dram_tensor("x8", (N, D), fp8, kind="ExternalInput")
    xs8 = nc.dram_tensor("xs8", (N, JS), fp8, kind="ExternalInput")
    # xtl[nb, p, kt, n] = x[nb*128 + n, kt*128 + p] — phase-4 lhsT tiles, one
    # fully contiguous 1 MiB DMA per output row-block.
    xtl = nc.dram_tensor("xtl", (NB, P, DKT, P), bf16, kind="ExternalInput")
    out = nc.dram_tensor("out", (N, JS), f32, kind="ExternalOutput")
    # scratch for relaying 1/rowsum from partition layout to free-axis layout
    rsd = nc.dram_tensor("rsd", (SBI, P), f32)

    with tile.TileContext(nc) as tc:
        with (
            tc.tile_pool(name="psum", bufs=8, space="PSUM") as psum,
            tc.tile_pool(name="stats", bufs=8) as stats,
            tc.tile_pool(name="singles", bufs=1) as singles,
            tc.tile_pool(name="ptp", bufs=DKT) as ptp,
        ):
            ident = singles.tile([P, P], bf16)
            make_identity(nc, ident)
            pt = [ptp.tile([P, JS], bf16, tag="pt", name=f"pt{i}") for i in range(DKT)]

            with (
                tc.tile_pool(name="big", bufs=5) as big,
                tc.tile_pool(name="xsp", bufs=NKT // 2) as xsp,
                tc.tile_pool(name="rhsp", bufs=6) as rhsp,
                tc.tile_pool(name="xtp", bufs=5) as xtp,
                tc.tile_pool(name="outp", bufs=3) as outp,
            ):
                a_s = [
                    big.tile([P, D], f32, tag="big", name=f"a_s{i}")
                    for i in range(SBI)
                ]
                pmax = [
                    stats.tile([P, NCH], f32, tag="pmax", name=f"pmax{i}", bufs=4)
                    for i in range(SBI)
                ]
                if True:
                    # ---------------- Phase 1: Gram row-shard ----------------
                    # fp8 DoubleRow: each matmul contracts a k-PAIR of 128-row
                    # tiles (virtual 128x256 array, 2 fp8 weights per cell).
                    # One explicit LDWEIGHTS per (k-pair, bi) serves TWO
                    # matmuls (1024-wide d2 pass-groups, 8 PSUM banks =
                    # 4 bi x 2 halves); nosync ordering edges keep the PE
                    # stream LDW,MM,MM,LDW,... so the 256-column weight load
                    # hides under the matmul stream instead of bounding it.
                    from concourse.tile import add_dep_helper

                    NKP = NKT // 2
                    NPG = 4  # d2 pass-groups of 1024
                    xst = [
                        xsp.tile([P, 2, JS], fp8, tag="xs", name=f"xs_{k}")
                        for k in range(NKP)
                    ]
                    last_pss = [None] * SBI
                    prev_pe = None
                    for pg in range(NPG):
                        pss = [
                            psum.tile([P, JS], f32, tag="ps", name=f"ps1_{pg}_{i}")
                            for i in range(2 * SBI)
                        ]
                        g0 = pg * 2 * JS
                        for kp in range(NKP):
                            r0 = kp * 2 * P
                            if pg == 0:
                                nc.gpsimd.dma_start(
                                    out=xst[kp],
                                    in_=xs8[r0 : r0 + 2 * P, :].rearrange(
                                        "(ko p) m -> p ko m", p=P
                                    ),
                                )
                            rt = rhsp.tile(
                                [P, 2, 2 * JS], fp8, tag="rt", name=f"rt_{pg}_{kp}"
                            )
                            nc.sync.dma_start(
                                out=rt,
                                in_=x8[r0 : r0 + 2 * P, g0 : g0 + 2 * JS].rearrange(
                                    "(ko p) d -> p ko d", p=P
                                ),
                            )
                            for bi in range(SBI):
                                w_ap = xst[kp][:, :, bi * P : (bi + 1) * P]
                                ldw = nc.tensor.ldweights(
                                    w_ap, perf_mode=mybir.MatmulPerfMode.DoubleRow
                                )
                                if prev_pe is not None:
                                    add_dep_helper(
                                        ldw.ins, prev_pe, False,
                                        reason="keep PE ldw/mm order",
                                    )
                                prev_pe = ldw.ins
                                for h in range(2):
                                    mmi = nc.tensor.matmul(
                                        pss[bi * 2 + h],
                                        w_ap,
                                        rt[:, :, h * JS : (h + 1) * JS],
                                        start=(kp == 0),
                                        stop=(kp == NKP - 1),
                                        perf_mode=mybir.MatmulPerfMode.DoubleRow,
                                    )
                                    mmi.ins.ldweights = False
                                    add_dep_helper(
                                        mmi.ins, prev_pe, False,
                                        reason="keep PE ldw/mm order",
                                    )
                                    prev_pe = mmi.ins
                        for bi in range(SBI):
                            for h in range(2):
                                ch = pg * 2 + h
                                nc.vector.reduce_max(
                                    out=pmax[bi][:, ch : ch + 1],
                                    in_=pss[bi * 2 + h],
                                    axis=mybir.AxisListType.X,
                                )
                        for bi in range(SBI):
                            for h in range(2):
                                ch = pg * 2 + h
                                if ch < NCH - 1:
                                    nc.vector.tensor_copy(
                                        out=a_s[bi][:, ch * JS : (ch + 1) * JS],
                                        in_=pss[bi * 2 + h],
                                    )
                                else:
                                    last_pss[bi] = pss[bi * 2 + h]

                # ------------- Phase 2+3: softmax rows, transpose -------------
                # exp is chunked so PE transposes chase the ACT engine instead
                # of waiting for whole rows; the 1/rowsum scale is deferred to
                # the phase-4 PSUM evacuation (column scales commute through
                # the matmul, and applying them in f32 at the end is exact).
                TPC = JS // P  # transposes per exp chunk
                # prefetch the first phase-4 lhsT blocks; the in-order sync
                # queue starts these the moment phase 1's stream drains, so
                # they land during the softmax/transposes.
                xtts = {}
                for nb in range(4):
                    xtts[nb] = xtp.tile([P, DKT, P], bf16, tag="xt", name=f"xtt{nb}")
                    nc.sync.dma_start(out=xtts[nb], in_=xtl[nb])
                for bi in range(SBI):
                    m = stats.tile([P, 1], f32, tag="m", name=f"m{bi}")
                    nc.vector.reduce_max(out=m, in_=pmax[bi], axis=mybir.AxisListType.X)
                    negm = stats.tile([P, 1], f32, tag="negm", name=f"negm{bi}")
                    nc.vector.tensor_scalar_mul(out=negm, in0=m, scalar1=-1.0)
                    pacc = stats.tile([P, NCH], f32, tag="pacc", name=f"pacc{bi}", bufs=4)
                    p_s = big.tile([P, D], bf16, tag="big", name=f"p_s{bi}")
                    for c in [NCH - 1] + list(range(NCH - 1)):
                        c0 = c * JS
                        src_ap = (
                            last_pss[bi] if c == NCH - 1 else a_s[bi][:, c0 : c0 + JS]
                        )
                        nc.scalar.activation(
                            out=p_s[:, c0 : c0 + JS],
                            in_=src_ap,
                            func=mybir.ActivationFunctionType.Exp,
                            bias=negm,
                            scale=1.0,
                            accum_out=pacc[:, c : c + 1],
                        )
                        for t in range(c * TPC, (c + 1) * TPC):
                            tp = psum.tile([P, P], bf16, tag="ps", name=f"tp{bi}_{t}")
                            nc.tensor.transpose(tp, p_s[:, t * P : (t + 1) * P], ident)
                            nc.vector.tensor_copy(
                                out=pt[t][:, bi * P : (bi + 1) * P], in_=tp
                            )
                    ssum = stats.tile([P, 1], f32, tag="ssum", name=f"ssum{bi}")
                    nc.vector.reduce_sum(out=ssum, in_=pacc, axis=mybir.AxisListType.X)
                    rs = stats.tile([P, 1], f32, tag="rs", name=f"rs{bi}")
                    nc.vector.reciprocal(out=rs, in_=ssum)
                    nc.gpsimd.dma_start(out=rsd[bi], in_=rs)
                # broadcast [512] reciprocals across partitions: [128, SBI*P]
                rsb = singles.tile([P, SBI, P], f32, name="rsb")
                nc.gpsimd.dma_start(
                    out=rsb,
                    in_=bass.AP(tensor=rsd, offset=0, ap=[[0, P], [P, SBI], [1, P]]),
                )

                # ---------------- Phase 4: c_S = x @ b_S ----------------
                for nb in range(NB):
                    if nb in xtts:
                        xtt = xtts.pop(nb)
                    else:
                        xtt = xtp.tile([P, DKT, P], bf16, tag="xt", name=f"xtt{nb}")
                        nc.sync.dma_start(out=xtt, in_=xtl[nb])
                    ps = psum.tile([P, JS], f32, tag="ps", name=f"ps4_{nb}")
                    for kt in range(DKT):
                        nc.tensor.matmul(
                            ps,
                            xtt[:, kt, :],
                            pt[kt],
                            start=(kt == 0),
                            stop=(kt == DKT - 1),
                        )
                    ot = outp.tile([P, JS], f32, tag="ot", name=f"ot{nb}")
                    nc.vector.tensor_mul(
                        out=ot, in0=ps, in1=rsb.rearrange("p a b -> p (a b)")
                    )
                    nc.sync.dma_start(out=out[nb * P : (nb + 1) * P, :], in_=ot)
    nc.finalize()
    return nc


def _get_nc():
    global _nc_cache
    if _nc_cache is None:
        _nc_cache = _build()
    return _nc_cache


def kernel(x):
    import ml_dtypes
    from concourse.bass_utils import run_bass_kernel_spmd

    x = np.asarray(x, dtype=np.float32)
    assert x.shape == (N, D)
    xb = x.astype(ml_dtypes.bfloat16)
    x8 = x.astype(ml_dtypes.float8_e4m3)
    # xtl[nb, p, kt, n] = x[nb*128 + n, kt*128 + p]
    xtl = np.ascontiguousarray(
        xb.reshape(NB, P, DKT, P).transpose(0, 3, 2, 1)
    )
    in_maps = [
        {
            "x8": x8,
            "xs8": np.ascontiguousarray(x8[:, i * JS : (i + 1) * JS]),
            "xtl": xtl,
        }
        for i in range(NCORES)
    ]
    nc = _get_nc()
    res = run_bass_kernel_spmd(nc, in_maps, core_ids=list(range(NCORES)))
    out = np.concatenate([r["out"] for r in res.results], axis=1)
    return out


# revision 17
# speedup vs baseline: 2.7244x; 1.0352x over previous
"""Distributed Trainium2 kernel for: a = x.T @ x ; b = softmax(a, axis=0) ; c = x @ b.

Strategy (8 NeuronCores, no collectives — embarrassingly parallel column shard):
  Core i owns output columns S_i = [512*i, 512*(i+1)).
  Since a is symmetric, the column-softmax stats for columns S_i are the row
  stats of the row shard a[S_i, :], which reduce along the free axis on-chip.

  Phase 1: a_S = x[:, S].T @ x          [512, 4096]   (Gram row-shard, f32 PSUM)
  Phase 2: P = row_softmax(a_S)         (= b[:, S].T, computed in f32)
  Phase 3: PE-transpose P -> b_S        [4096, 512]
  Phase 4: c[:, S] = x @ b_S            via lhsT = x.T tiles (host-pretiled)

Matmul operands are bf16 (1 cycle/row on the PE — 4-byte fp32 operands stream
at half rate) with fp32 PSUM accumulation; the softmax stats run in fp32.
"""

import numpy as np

N, D, P = 8192, 4096, 128
NCORES = 8
JS = D // NCORES          # 512 columns per core
SBI = JS // P             # 4 shard row-blocks of a_S
NKT = N // P              # 64 contraction tiles for the Gram
NCH = D // JS             # 8 chunks of 512 over the Gram free dim
DKT = D // P              # 32 contraction tiles for phase 4
NB = N // P               # 64 output row blocks

_nc_cache = None


def _build():
    import concourse.bass as bass
    import concourse.mybir as mybir
    import concourse.tile as tile
    from concourse import bacc
    from concourse.masks import make_identity

    f32 = mybir.dt.float32
    bf16 = mybir.dt.bfloat16
    fp8 = mybir.dt.float8e4

    nc = bacc.Bacc("TRN2", target_bir_lowering=False)
    # fp8 e4m3 copies of x feed the Gram phase (DoubleRow, 2x MACs/cycle);
    # the Gram only feeds a saturated softmax, so fp8 precision is ample.
    x8 = nc.dram_tensor("x8", (N, D), fp8, kind="ExternalInput")
    xs8 = nc.dram_tensor("xs8", (N, JS), fp8, kind="ExternalInput")
    # xtl[nb, p, kt, n] = x[nb*128 + n, kt*128 + p] — phase-4 lhsT tiles, one
    # fully contiguous 1 MiB DMA per output row-block.
    xtl = nc.dram_tensor("xtl", (NB, P, DKT, P), bf16, kind="ExternalInput")
    out = nc.dram_tensor("out", (N, JS), f32, kind="ExternalOutput")
    # scratch for relaying 1/rowsum from partition layout to free-axis layout
    rsd = nc.dram_tensor("rsd", (SBI, P), f32)

    with tile.TileContext(nc) as tc:
        with (
            tc.tile_pool(name="psum", bufs=8, space="PSUM") as psum,
            tc.tile_pool(name="stats", bufs=8) as stats,
            tc.tile_pool(name="singles", bufs=1) as singles,
            tc.tile_pool(name="ptp", bufs=DKT) as ptp,
        ):
            ident = singles.tile([P, P], bf16)
            make_identity(nc, ident)
            pt = [ptp.tile([P, JS], bf16, tag="pt", name=f"pt{i}") for i in range(DKT)]

            with (
                tc.tile_pool(name="big", bufs=5) as big,
                tc.tile_pool(name="xsp", bufs=NKT // 2) as xsp,
                tc.tile_pool(name="rhsp", bufs=12) as rhsp,
                tc.tile_pool(name="xtp", bufs=5) as xtp,
                tc.tile_pool(name="outp", bufs=3) as outp,
            ):
                a_s = [
                    big.tile([P, D], f32, tag="big", name=f"a_s{i}")
                    for i in range(SBI)
                ]
                pmax = [
                    stats.tile([P, NCH], f32, tag="pmax", name=f"pmax{i}", bufs=4)
                    for i in range(SBI)
                ]
                if True:
                    # ---------------- Phase 1: Gram row-shard ----------------
                    # fp8 DoubleRow: each matmul contracts a k-PAIR of 128-row
                    # tiles (virtual 128x256 array, 2 fp8 weights per cell).
                    NKP = NKT // 2
                    xst = [
                        xsp.tile([P, 2, JS], fp8, tag="xs", name=f"xs_{k}")
                        for k in range(NKP)
                    ]
                    for ch in range(NCH):
                        pss = [
                            psum.tile([P, JS], f32, tag="ps", name=f"ps1_{ch}_{i}")
                            for i in range(SBI)
                        ]
                        c0 = ch * JS
                        for kp in range(NKP):
                            r0 = kp * 2 * P
                            if ch == 0:
                                nc.gpsimd.dma_start(
                                    out=xst[kp],
                                    in_=xs8[r0 : r0 + 2 * P, :].rearrange(
                                        "(ko p) m -> p ko m", p=P
                                    ),
                                )
                            rt = rhsp.tile([P, 2, JS], fp8, tag="rt", name=f"rt_{ch}_{kp}")
                            nc.sync.dma_start(
                                out=rt,
                                in_=x8[r0 : r0 + 2 * P, c0 : c0 + JS].rearrange(
                                    "(ko p) d -> p ko d", p=P
                                ),
                            )
                            for bi in range(SBI):
                                nc.tensor.matmul(
                                    pss[bi],
                                    xst[kp][:, :, bi * P : (bi + 1) * P],
                                    rt,
                                    start=(kp == 0),
                                    stop=(kp == NKP - 1),
                                    perf_mode=mybir.MatmulPerfMode.DoubleRow,
                                )
                        for bi in range(SBI):
                            nc.vector.reduce_max(
                                out=pmax[bi][:, ch : ch + 1],
                                in_=pss[bi],
                                axis=mybir.AxisListType.X,
                            )
                        if ch < NCH - 1:
                            for bi in range(SBI):
                                nc.vector.tensor_copy(
                                    out=a_s[bi][:, c0 : c0 + JS], in_=pss[bi]
                                )
                        else:
                            last_pss = pss  # last chunk exps straight from PSUM

                # ------------- Phase 2+3: softmax rows, transpose -------------
                # exp is chunked so PE transposes chase the ACT engine instead
                # of waiting for whole rows; the 1/rowsum scale is deferred to
                # the phase-4 PSUM evacuation (column scales commute through
                # the matmul, and applying them in f32 at the end is exact).
                TPC = JS // P  # transposes per exp chunk
                # prefetch the first phase-4 lhsT blocks; the in-order sync
                # queue starts these the moment phase 1's stream drains, so
                # they land during the softmax/transposes.
                xtts = {}
                for nb in range(4):
                    xtts[nb] = xtp.tile([P, DKT, P], bf16, tag="xt", name=f"xtt{nb}")
                    nc.sync.dma_start(out=xtts[nb], in_=xtl[nb])
                for bi in range(SBI):
                    m = stats.tile([P, 1], f32, tag="m", name=f"m{bi}")
                    nc.vector.reduce_max(out=m, in_=pmax[bi], axis=mybir.AxisListType.X)
                    negm = stats.tile([P, 1], f32, tag="negm", name=f"negm{bi}")
                    nc.vector.tensor_scalar_mul(out=negm, in0=m, scalar1=-1.0)
                    pacc = stats.tile([P, NCH], f32, tag="pacc", name=f"pacc{bi}", bufs=4)
                    p_s = big.tile([P, D], bf16, tag="big", name=f"p_s{bi}")
                    for c in [NCH - 1] + list(range(NCH - 1)):
                        c0 = c * JS
                        src_ap = (
                            last_pss[bi] if c == NCH - 1 else a_s[bi][:, c0 : c0 + JS]
                        )
                        nc.scalar.activation(
                            out=p_s[:, c0 : c0 + JS],
                            in_=src_ap,
                            func=mybir.ActivationFunctionType.Exp,
                            bias=negm,
                            scale=1.0,
                            accum_out=pacc[:, c : c + 1],
                        )
                        for t in range(c * TPC, (c + 1) * TPC):
                            tp = psum.tile([P, P], bf16, tag="ps", name=f"tp{bi}_{t}")
                            nc.tensor.transpose(tp, p_s[:, t * P : (t + 1) * P], ident)
                            nc.vector.tensor_copy(
                                out=pt[t][:, bi * P : (bi + 1) * P], in_=tp
                            )
                    ssum = stats.tile([P, 1], f32, tag="ssum", name=f"ssum{bi}")
                    nc.vector.reduce_sum(out=ssum, in_=pacc, axis=mybir.AxisListType.X)
                    rs = stats.tile([P, 1], f32, tag="rs", name=f"rs{bi}")
                    nc.vector.reciprocal(out=rs, in_=ssum)
                    nc.gpsimd.dma_start(out=rsd[bi], in_=rs)
                # broadcast [512] reciprocals across partitions: [128, SBI*P]
                rsb = singles.tile([P, SBI, P], f32, name="rsb")
                nc.gpsimd.dma_start(
                    out=rsb,
                    in_=bass.AP(tensor=rsd, offset=0, ap=[[0, P], [P, SBI], [1, P]]),
                )

                # ---------------- Phase 4: c_S = x @ b_S ----------------
                for nb in range(NB):
                    if nb in xtts:
                        xtt = xtts.pop(nb)
                    else:
                        xtt = xtp.tile([P, DKT, P], bf16, tag="xt", name=f"xtt{nb}")
                        nc.sync.dma_start(out=xtt, in_=xtl[nb])
                    ps = psum.tile([P, JS], f32, tag="ps", name=f"ps4_{nb}")
                    for kt in range(DKT):
                        nc.tensor.matmul(
                            ps,
                            xtt[:, kt, :],
                            pt[kt],
                            start=(kt == 0),
                            stop=(kt == DKT - 1),
                        )
                    ot = outp.tile([P, JS], f32, tag="ot", name=f"ot{nb}")
                    nc.vector.tensor_mul(
                        out=ot, in0=ps, in1=rsb.rearrange("p a b -> p (a b)")
                    )
                    nc.sync.dma_start(out=out[nb * P : (nb + 1) * P, :], in_=ot)
    nc.finalize()
    return nc


def _get_nc():
    global _nc_cache
    if _nc_cache is None:
        _nc_cache = _build()
    return _nc_cache


def kernel(x):
    import ml_dtypes
    from concourse.bass_utils import run_bass_kernel_spmd

    x = np.asarray(x, dtype=np.float32)
    assert x.shape == (N, D)
    xb = x.astype(ml_dtypes.bfloat16)
    x8 = x.astype(ml_dtypes.float8_e4m3)
    # xtl[nb, p, kt, n] = x[nb*128 + n, kt*128 + p]
    xtl = np.ascontiguousarray(
        xb.reshape(NB, P, DKT, P).transpose(0, 3, 2, 1)
    )
    in_maps = [
        {
            "x8": x8,
            "xs8": np.ascontiguousarray(x8[:, i * JS : (i + 1) * JS]),
            "xtl": xtl,
        }
        for i in range(NCORES)
    ]
    nc = _get_nc()
    res = run_bass_kernel_spmd(nc, in_maps, core_ids=list(range(NCORES)))
    out = np.concatenate([r["out"] for r in res.results], axis=1)
    return out
